# revision 1
# baseline (speedup 1.0000x reference)
"""DegreeQuantileConverter Trainium2 kernel.

deg (B,S,1) f32 -> out (B,S,12) f32 = log(w + 1e-30) where w are the
piecewise-linear interpolation weights of deg onto the quantile grid
q = [0,1,2,4,...,1024], with rows where deg >= 1024 forced to w = 1.

Math: with c_j = clip((d - q_j)/(q_{j+1}-q_j), 0, 1) for j=0..10 the
weights telescope:  w_0 = 1-c_0, w_j = c_{j-1}-c_j, w_11 = c_10.
Since q_j/(q_{j+1}-q_j) == 1 for j>=1, z_j = d*inv_j - 1 (inv_j a power
of two), which keeps every value bit-identical to the reference's
(d-lo)/(hi-lo) path.  The deg>=1024 all-ones override is applied on the
host (cheap boolean mask on the gathered result).

Sharding: batch 128 -> 16 rows per core x 8 cores, each core sees its
shard as [128 partitions x 2048 cols]; output is written channel-major
[128, 12, 2048] per core and re-interleaved on the host.
"""

import numpy as np

import concourse.bacc as bacc
import concourse.mybir as mybir
import concourse.tile as tile
from concourse.bass_utils import run_bass_kernel_spmd

AF = mybir.ActivationFunctionType
OP = mybir.AluOpType
F32 = mybir.dt.float32
F16 = mybir.dt.float16

B, S, K = 128, 16384, 12
NCORES = 8
P = 128
ELEMS = (B // NCORES) * S      # 262144 per core
COLS = ELEMS // P              # 2048
F = 1024                       # free-dim tile size
NT = COLS // F                 # 2 tiles per core

QL = [0.0, 1.0, 2.0, 4.0, 8.0, 16.0, 32.0, 64.0, 128.0, 256.0, 512.0, 1024.0]
INV = [1.0] + [1.0 / (QL[j + 1] - QL[j]) for j in range(1, 11)]

# The device Ln table is only accurate for inputs in ~[1e-19, 1e19], but we
# need ln(w + 1e-30) with w in {0} u [3e-8, 1].  So compute
# Ln(w * 2^50 + 1e-30 * 2^50) on device (inputs then span [1.1e-15, 1.1e15])
# and subtract 50*ln2 on the host.
LN_SCALE = float(np.float32(2.0**50))
LN_BIAS = float(np.float32(np.float64(np.float32(1e-30)) * 2.0**50))
LN_OFFSET = np.float32(50.0 * np.log(np.float64(2.0)))

# channels whose affine+relu (y_j = relu(d*inv_j - 1)) runs on ACT; the
# rest compute z on DVE (GPSIMD is ~20x slower than DVE for fp32
# elementwise and throttles concurrent DVE via shared SBUF ports — avoid).
ACT_Z_CHANNELS = frozenset(range(1, 9))


def build_program():
    nc = bacc.Bacc("TRN2", target_bir_lowering=False, debug=False, num_devices=NCORES)
    # register activation-bias constants (only 0.0/1.0 are pre-registered)
    for name, val in (("lnbias", LN_BIAS), ("negone", -1.0)):
        ct = nc.alloc_sbuf_tensor(f"const-float32-{name}", [128, 1], F32)
        nc.gpsimd.memset(ct.ap(), val)
        nc.const_aps.aps[(F32, val)] = ct.ap()
    nc.all_engine_barrier()
    d_ext = nc.declare_dram_parameter("degrees", [P, COLS], F32, isOutput=False)
    out_ext = nc.declare_dram_parameter("out", [P, K, COLS], F16, isOutput=True)

    with tile.TileContext(nc) as tc:
        with (
            tc.tile_pool(name="dp", bufs=2) as dp,
            tc.tile_pool(name="cp", bufs=2) as cp,
            tc.tile_pool(name="sw", bufs=1) as sw,
            tc.tile_pool(name="so", bufs=2) as so,
        ):
            # dummy Ln before anything else: pulls the ACT table load for the
            # Ln set into the preamble window, and keeps Relu (present in
            # every set) from loading a different set first.
            dummy = dp.tile([P, 1], F32, tag="dummy")
            nc.gpsimd.memset(dummy[:], 1.0)
            nc.scalar.activation(dummy[:], dummy[:], AF.Ln, bias=LN_BIAS, scale=LN_SCALE)

            for t in range(NT):
                d = dp.tile([P, F], F32, tag="d")
                nc.sync.dma_start(out=d[:], in_=d_ext[:, t * F : (t + 1) * F])

                stg_a = sw.tile([P, 6 * F], F32, tag="stg_a")
                stg_b = sw.tile([P, 6 * F], F32, tag="stg_b")
                o16_a = so.tile([P, 6 * F], F16, tag="o16_a")
                o16_b = so.tile([P, 6 * F], F16, tag="o16_b")

                def stg_slice(j):
                    return (
                        stg_a[:, j * F : (j + 1) * F]
                        if j < 6
                        else stg_b[:, (j - 6) * F : (j - 5) * F]
                    )

                # alternate channel order per tile so the final Ln+DMA tail
                # of the last tile is the small (10,11) ... (0,1) reversal
                rev = t == NT - 1
                ch_order = range(10, -1, -1) if rev else range(11)

                # ln groups + their DMAs, fired inline as soon as every
                # w-channel of the group has been emitted, so the output
                # stream starts while relus are still running
                groups = [(0, 2), (2, 4), (4, 6), (6, 10), (10, 12)]
                done_w = set()

                def flush_groups():
                    for j0, j1 in groups:
                        if (j0, j1) in done_w:
                            continue
                        if not all(j in done_w for j in range(j0, j1)):
                            continue
                        done_w.add((j0, j1))
                        sl = (
                            stg_a[:, j0 * F : j1 * F]
                            if j1 <= 6
                            else stg_b[:, (j0 - 6) * F : (j1 - 6) * F]
                        )
                        dst = (
                            o16_a[:, j0 * F : j1 * F]
                            if j1 <= 6
                            else o16_b[:, (j0 - 6) * F : (j1 - 6) * F]
                        )
                        nc.scalar.activation(
                            dst, sl, AF.Ln, bias=LN_BIAS, scale=LN_SCALE
                        )
                        nc.sync.dma_start(
                            out=out_ext[:, j0:j1, t * F : (t + 1) * F],
                            in_=dst.rearrange("p (j f) -> p j f", j=j1 - j0),
                        )

                c = {}
                for j in ch_order:
                    if j == 10:
                        # c_10 goes straight into the ch11 staging slot
                        cj = stg_b[:, 5 * F : 6 * F]
                    else:
                        cj_t = cp.tile([P, F], F32, tag=f"c{j}")
                        cj = cj_t[:]
                    if j == 0:
                        nc.vector.tensor_scalar(cj, d[:], 0.0, 1.0, OP.max, OP.min)
                    elif j in ACT_Z_CHANNELS:
                        nc.scalar.activation(cj, d[:], AF.Relu, bias=-1.0, scale=INV[j])
                        nc.vector.tensor_scalar(cj, cj, 1.0, None, OP.min)
                    else:
                        nc.vector.tensor_scalar(
                            cj, d[:], INV[j], 1.0, OP.mult, OP.subtract
                        )
                        nc.vector.tensor_scalar(cj, cj, 0.0, 1.0, OP.max, OP.min)
                    c[j] = cj
                    if j == 10:
                        done_w.add(11)  # w_11 = c_10, already in its slot
                    if j == 0:
                        # w_0 = 1 - c_0
                        nc.vector.tensor_scalar(
                            stg_slice(0), c[0], -1.0, 1.0, OP.mult, OP.add
                        )
                        done_w.add(0)
                    # emit diffs as soon as both operands exist
                    if not rev and j > 0:
                        nc.vector.tensor_tensor(
                            stg_slice(j), c[j - 1], c[j], OP.subtract
                        )
                        done_w.add(j)
                    if rev and j + 1 in c:
                        nc.vector.tensor_tensor(
                            stg_slice(j + 1), c[j], c[j + 1], OP.subtract
                        )
                        done_w.add(j + 1)
                    flush_groups()
    nc.compile()
    return nc


_CACHE = {}
RUN_KWARGS = {}  # test harness can set e.g. {"trace": True} for profiling


def kernel(degrees, quantile_values):
    q = np.asarray(quantile_values, dtype=np.float32)
    assert np.array_equal(q, np.array(QL, dtype=np.float32)), "unexpected quantile grid"

    deg = np.ascontiguousarray(np.asarray(degrees, dtype=np.float32)[..., 0])  # (B,S)
    shards = deg.reshape(NCORES, P, COLS)

    if "nc" not in _CACHE:
        _CACHE["nc"] = build_program()
    nc = _CACHE["nc"]

    in_maps = [{"degrees": np.ascontiguousarray(shards[i])} for i in range(NCORES)]
    res = run_bass_kernel_spmd(nc, in_maps, list(range(NCORES)), **RUN_KWARGS)
    _CACHE["last_result"] = res
    outs = np.stack([res.results[i]["out"] for i in range(NCORES)])  # (8,128,12,2048)

    full = (
        outs.transpose(0, 1, 3, 2)  # (8,128,2048,12) — element order, channel last
        .reshape(B, S, K)
        .astype(np.float32, copy=True)
    )
    full -= LN_OFFSET
    full[deg >= np.float32(1024.0)] = np.float32(0.0)
    return full



# revision 3
# speedup vs baseline: 2.0655x; 2.0655x over previous
"""DegreeQuantileConverter Trainium2 kernel (exponent-trick edition).

deg (B,S,1) f32 -> out (B,S,12) f32 = log(w + 1e-30) where w are the
piecewise-linear interpolation weights of deg onto the quantile grid
q = [0,1,2,4,...,1024], with rows where deg >= 1024 forced to w = 1.

Because the grid is powers of two, the bin index is the f32 exponent
field of max(d,1) and pos = (d-lo)/(hi-lo) is an exact bit manipulation:
    e_bits = bits(max(d,1)) & 0x7F800000          # lo = 2^e
    scale  = bitcast(0x7F000000 - e_bits)         # 2^-e, exact
    p1     = d * scale                            # in [0,2)
    npos   = (p1 >= 1) - p1                       # = -pos, exact frac
Only two channels of the 12 are ever non-constant: w[idx] = 1-pos and
w[idx+1] = pos; everything else is log(1e-30).  The device computes the
two log-weights:
    la = Ln(2^50*npos + 2^50)        = ln(1-pos) + 50*ln2
    lb = Ln(-2^50*npos + 2^50*1e-30) = ln(pos + 1e-30) + 50*ln2
(the 2^50 scaling keeps the Ln table input inside its accurate range;
the host subtracts 50*ln2).  The host scatters la/lb into a
log(1e-30)-filled (B,S,12) array at channels idx/idx+1, where idx is
the same exponent extraction done in numpy, and zeroes rows d >= 1024.

Sharding: batch 128 -> 16 rows per core x 8 cores, each core sees its
shard as [128 partitions x 2048 cols].
"""

import numpy as np

import concourse.bacc as bacc
import concourse.mybir as mybir
import concourse.tile as tile
from concourse.bass_utils import run_bass_kernel_spmd

AF = mybir.ActivationFunctionType
OP = mybir.AluOpType
F32 = mybir.dt.float32
F16 = mybir.dt.float16
I32 = mybir.dt.int32

B, S, K = 128, 16384, 12
NCORES = 8
P = 128
ELEMS = (B // NCORES) * S      # 262144 per core
COLS = ELEMS // P              # 2048
F = 512                        # free-dim tile size
NT = COLS // F                 # tiles per core

QL = [0.0, 1.0, 2.0, 4.0, 8.0, 16.0, 32.0, 64.0, 128.0, 256.0, 512.0, 1024.0]

# Ln table is only accurate for inputs in ~[1e-19, 1e19]; compute
# Ln(w * 2^50 + 1e-30 * 2^50) on device and subtract 50*ln2 on the host.
LN_SCALE = float(np.float32(2.0**50))
LN_BIAS = float(np.float32(np.float64(np.float32(1e-30)) * 2.0**50))
BIAS_LA = float(np.float32(2.0**50))
LN_OFFSET = np.float32(50.0 * np.log(np.float64(2.0)))
LOG_EPS = np.float32(np.log(np.float64(np.float32(1e-30))))  # -69.07755

EXP_MASK = 0x7F800000
SCALE_BASE = 0x7F000000  # bits(2^127); SCALE_BASE - e_bits = bits(2^-e)


def build_program():
    nc = bacc.Bacc("TRN2", target_bir_lowering=False, debug=False, num_devices=NCORES)
    # register activation-bias constants (only 0.0/1.0 are pre-registered)
    for name, val in (("lnbias", LN_BIAS), ("biasla", BIAS_LA)):
        ct = nc.alloc_sbuf_tensor(f"const-float32-{name}", [128, 1], F32)
        nc.gpsimd.memset(ct.ap(), val)
        nc.const_aps.aps[(F32, val)] = ct.ap()
    nc.all_engine_barrier()
    d_ext = nc.declare_dram_parameter("degrees", [P, COLS], F32, isOutput=False)
    lab_ext = nc.declare_dram_parameter("lab", [P, 2, COLS], F16, isOutput=True)

    with tile.TileContext(nc) as tc:
        with (
            tc.tile_pool(name="dp", bufs=2) as dp,
            tc.tile_pool(name="wp", bufs=2) as wp,
            tc.tile_pool(name="op", bufs=2) as op,
        ):
            # dummy Ln first: pulls the ACT table load into the preamble
            dummy = dp.tile([P, 1], F32, tag="dummy")
            nc.gpsimd.memset(dummy[:], 1.0)
            nc.scalar.activation(dummy[:], dummy[:], AF.Ln, bias=LN_BIAS, scale=LN_SCALE)

            for t in range(NT):
                d = dp.tile([P, F], F32, tag="d")
                nc.sync.dma_start(out=d[:], in_=d_ext[:, t * F : (t + 1) * F])

                dc = wp.tile([P, F], F32, tag="dc")
                sb = wp.tile([P, F], I32, tag="sb")
                p1 = wp.tile([P, F], F32, tag="p1")
                npos = wp.tile([P, F], F32, tag="npos")
                lab = op.tile([P, 2 * F], F16, tag="lab")

                # dc = max(d, 1);  eb = bits(dc) & EXP_MASK (into sb's slot)
                nc.vector.tensor_scalar(dc[:], d[:], 1.0, None, OP.max)
                nc.vector.tensor_scalar(sb[:], dc[:].bitcast(I32), EXP_MASK, None, OP.bitwise_and)
                # sb = -eb + SCALE_BASE  (= bits(2^-e))
                nc.vector.tensor_scalar(sb[:], sb[:], -1, SCALE_BASE, OP.mult, OP.add)
                # p1 = d * 2^-e in [0,2);  npos = (p1>=1) - p1 = -pos
                nc.vector.tensor_tensor(p1[:], d[:], sb[:].bitcast(F32), OP.mult)
                nc.vector.scalar_tensor_tensor(npos[:], p1[:], 1.0, p1[:], OP.is_ge, OP.subtract)
                # la = Ln(2^50*(1-pos)); lb = Ln(2^50*pos + 2^50*1e-30)
                nc.scalar.activation(lab[:, :F], npos[:], AF.Ln, bias=BIAS_LA, scale=LN_SCALE)
                nc.scalar.activation(lab[:, F:], npos[:], AF.Ln, bias=LN_BIAS, scale=-LN_SCALE)

                nc.sync.dma_start(
                    out=lab_ext[:, :, t * F : (t + 1) * F],
                    in_=lab[:].rearrange("p (c f) -> p c f", c=2),
                )
    nc.compile()
    return nc


_CACHE = {}
RUN_KWARGS = {}  # test harness can set e.g. {"trace": True} for profiling


def kernel(degrees, quantile_values):
    q = np.asarray(quantile_values, dtype=np.float32)
    assert np.array_equal(q, np.array(QL, dtype=np.float32)), "unexpected quantile grid"

    deg = np.ascontiguousarray(np.asarray(degrees, dtype=np.float32)[..., 0])  # (B,S)
    shards = deg.reshape(NCORES, P, COLS)

    if "nc" not in _CACHE:
        _CACHE["nc"] = build_program()
    nc = _CACHE["nc"]

    in_maps = [{"degrees": np.ascontiguousarray(shards[i])} for i in range(NCORES)]
    res = run_bass_kernel_spmd(nc, in_maps, list(range(NCORES)), **RUN_KWARGS)
    _CACHE["last_result"] = res
    labs = np.stack([res.results[i]["lab"] for i in range(NCORES)])  # (8,128,2,2048)

    la = labs[:, :, 0, :].astype(np.float32).reshape(B, S) - LN_OFFSET
    lb = labs[:, :, 1, :].astype(np.float32).reshape(B, S) - LN_OFFSET

    # lo-edge channel: grid is [0, 2^0 .. 2^10], so channel = exponent+1 for
    # d >= 1 and 0 for d < 1; (bits>>23)-126 clipped to [0,10] gives both.
    idx = np.clip((deg.view(np.int32) >> 23) - 126, 0, 10).astype(np.int64)

    full = np.full((B, S, K), LOG_EPS, dtype=np.float32)
    np.put_along_axis(full, idx[..., None], la[..., None], axis=2)
    np.put_along_axis(full, idx[..., None] + 1, lb[..., None], axis=2)
    full[deg >= np.float32(1024.0)] = np.float32(0.0)
    return full


# revision 4
# speedup vs baseline: 2.1238x; 1.0282x over previous
"""DegreeQuantileConverter Trainium2 kernel (exponent-trick edition).

deg (B,S,1) f32 -> out (B,S,12) f32 = log(w + 1e-30) where w are the
piecewise-linear interpolation weights of deg onto the quantile grid
q = [0,1,2,4,...,1024], with rows where deg >= 1024 forced to w = 1.

Because the grid is powers of two, the bin index is the f32 exponent
field and pos = (d-lo)/(hi-lo) is an exact bit manipulation:
    e_bits = bits(d) & 0x7F800000                 # lo = 2^e
    scale  = bitcast(0x7F000000 - e_bits)         # 2^-e, exact
    p1     = d * scale                            # mantissa in [1,2)
    npos   = (p1 >= 1) - p1                       # = -pos, exact
Only two channels of the 12 are ever non-constant: w[idx] = 1-pos and
w[idx+1] = pos; everything else is log(1e-30).  The device computes the
two log-weights:
    la = Ln(2^50*npos + 2^50)        = ln(1-pos) + 50*ln2
    lb = Ln(-2^50*npos + 2^50*1e-30) = ln(pos + 1e-30) + 50*ln2
(the 2^50 scaling keeps the Ln table input inside its accurate range;
the host subtracts 50*ln2).  The host scatters la/lb into a
log(1e-30)-filled (B,S,12) array at channels idx/idx+1, where idx is
the same exponent extraction done in numpy, zeroes rows d >= 1024, and
patches the ~0.1% of elements with d < 1 (bin [0,1), whose pos = d
does not follow the exponent formula) with exact numpy logs.

Sharding: batch 128 -> 16 rows per core x 8 cores, each core sees its
shard as [128 partitions x 2048 cols].
"""

import numpy as np

import concourse.bacc as bacc
import concourse.mybir as mybir
import concourse.tile as tile
from concourse.bass_utils import run_bass_kernel_spmd

AF = mybir.ActivationFunctionType
OP = mybir.AluOpType
F32 = mybir.dt.float32
F16 = mybir.dt.float16
I32 = mybir.dt.int32

B, S, K = 128, 16384, 12
NCORES = 8
P = 128
ELEMS = (B // NCORES) * S      # 262144 per core
COLS = ELEMS // P              # 2048
F = 512                        # free-dim compute tile size
NT = COLS // F                 # compute tiles per core
FIN = 1024                     # input dma chunk size
NIN = COLS // FIN

QL = [0.0, 1.0, 2.0, 4.0, 8.0, 16.0, 32.0, 64.0, 128.0, 256.0, 512.0, 1024.0]

# Ln table is only accurate for inputs in ~[1e-19, 1e19]; compute
# Ln(w * 2^50 + 1e-30 * 2^50) on device and subtract 50*ln2 on the host.
LN_SCALE = float(np.float32(2.0**50))
LN_BIAS = float(np.float32(np.float64(np.float32(1e-30)) * 2.0**50))
BIAS_LA = float(np.float32(2.0**50))
LN_OFFSET = np.float32(50.0 * np.log(np.float64(2.0)))
LOG_EPS = np.float32(np.log(np.float64(np.float32(1e-30))))  # -69.07755

EXP_MASK = 0x7F800000
SCALE_BASE = 0x7F000000  # bits(2^127); SCALE_BASE - e_bits = bits(2^-e)


def build_program():
    nc = bacc.Bacc("TRN2", target_bir_lowering=False, debug=False, num_devices=NCORES)
    d_ext = nc.declare_dram_parameter("degrees", [P, COLS], F32, isOutput=False)
    lab_ext = nc.declare_dram_parameter("lab", [P, 2, COLS], F16, isOutput=True)

    with tile.TileContext(nc) as tc:
        with (
            tc.tile_pool(name="dp", bufs=1) as dp,
            tc.tile_pool(name="wp", bufs=2) as wp,
            tc.tile_pool(name="op", bufs=2) as op,
        ):
            # activation bias constants as tracked tiles (no global barrier)
            cb = dp.tile([P, 2], F32, tag="cb")
            nc.gpsimd.memset(cb[:, 0:1], LN_BIAS)
            nc.gpsimd.memset(cb[:, 1:2], BIAS_LA)
            bias_lb = cb[:, 0:1]
            bias_la = cb[:, 1:2]

            # dummy Ln first: pulls the ACT table load into the preamble
            dummy = dp.tile([P, 1], F32, tag="dummy")
            nc.gpsimd.memset(dummy[:], 1.0)
            nc.scalar.activation(dummy[:], dummy[:], AF.Ln, bias=bias_lb, scale=LN_SCALE)

            # whole-shard input lives in SBUF; DMA it in NIN chunks issued
            # from the (otherwise idle) Pool engine
            d = dp.tile([P, COLS], F32, tag="d")
            for c in range(NIN):
                nc.gpsimd.dma_start(
                    out=d[:, c * FIN : (c + 1) * FIN],
                    in_=d_ext[:, c * FIN : (c + 1) * FIN],
                )

            for t in range(NT):
                ds = d[:, t * F : (t + 1) * F]
                sb = wp.tile([P, F], I32, tag="sb")
                p1 = wp.tile([P, F], F32, tag="p1")
                npos = wp.tile([P, F], F32, tag="npos")
                lab = op.tile([P, 2 * F], F16, tag="lab")

                # sb = bits(d) & EXP_MASK;  sb = -sb + SCALE_BASE (= bits(2^-e))
                nc.vector.tensor_scalar(sb[:], ds.bitcast(I32), EXP_MASK, None, OP.bitwise_and)
                nc.vector.tensor_scalar(sb[:], sb[:], -1, SCALE_BASE, OP.mult, OP.add)
                # p1 = d * 2^-e;  npos = (p1>=1) - p1 = -pos
                nc.vector.tensor_tensor(p1[:], ds, sb[:].bitcast(F32), OP.mult)
                nc.vector.scalar_tensor_tensor(npos[:], p1[:], 1.0, p1[:], OP.is_ge, OP.subtract)
                # la = Ln(2^50*(1-pos)); lb = Ln(2^50*pos + 2^50*1e-30)
                nc.scalar.activation(lab[:, :F], npos[:], AF.Ln, bias=bias_la, scale=LN_SCALE)
                nc.scalar.activation(lab[:, F:], npos[:], AF.Ln, bias=bias_lb, scale=-LN_SCALE)

                nc.gpsimd.dma_start(
                    out=lab_ext[:, :, t * F : (t + 1) * F],
                    in_=lab[:].rearrange("p (c f) -> p c f", c=2),
                )
    nc.compile()
    return nc


_CACHE = {}
RUN_KWARGS = {}  # test harness can set e.g. {"trace": True} for profiling


def kernel(degrees, quantile_values):
    q = np.asarray(quantile_values, dtype=np.float32)
    assert np.array_equal(q, np.array(QL, dtype=np.float32)), "unexpected quantile grid"

    deg = np.ascontiguousarray(np.asarray(degrees, dtype=np.float32)[..., 0])  # (B,S)
    shards = deg.reshape(NCORES, P, COLS)

    if "nc" not in _CACHE:
        _CACHE["nc"] = build_program()
    nc = _CACHE["nc"]

    in_maps = [{"degrees": np.ascontiguousarray(shards[i])} for i in range(NCORES)]
    res = run_bass_kernel_spmd(nc, in_maps, list(range(NCORES)), **RUN_KWARGS)
    _CACHE["last_result"] = res
    labs = np.stack([res.results[i]["lab"] for i in range(NCORES)])  # (8,128,2,2048)

    la = labs[:, :, 0, :].astype(np.float32).reshape(B, S) - LN_OFFSET
    lb = labs[:, :, 1, :].astype(np.float32).reshape(B, S) - LN_OFFSET

    # bin [0,1): device exponent path doesn't apply; exact host values
    low = deg < np.float32(1.0)
    if low.any():
        dl = deg[low].astype(np.float64)
        la[low] = np.float32(np.log1p(-dl))
        lb[low] = np.float32(np.log(dl + np.float64(np.float32(1e-30))))

    # lo-edge channel: grid is [0, 2^0 .. 2^10], so channel = exponent+1 for
    # d >= 1 and 0 for d < 1; (bits>>23)-126 clipped to [0,10] gives both.
    idx = np.clip((deg.view(np.int32) >> 23) - 126, 0, 10).astype(np.int64)

    full = np.full((B, S, K), LOG_EPS, dtype=np.float32)
    np.put_along_axis(full, idx[..., None], la[..., None], axis=2)
    np.put_along_axis(full, idx[..., None] + 1, lb[..., None], axis=2)
    full[deg >= np.float32(1024.0)] = np.float32(0.0)
    return full


# revision 7
# speedup vs baseline: 2.3238x; 1.0942x over previous
"""DegreeQuantileConverter Trainium2 kernel (mantissa-trick edition).

deg (B,S,1) f32 -> out (B,S,12) f32 = log(w + 1e-30) where w are the
piecewise-linear interpolation weights of deg onto the quantile grid
q = [0,1,2,4,...,1024], with rows where deg >= 1024 forced to w = 1.

Because the grid is powers of two, for d >= 1 the interpolation position
inside its bin is exactly the f32 mantissa fraction:
    m   = bitcast((bits(d) & 0x7FFFFF) | 0x3F800000)   # in [1,2)
    pos = m - 1,  1-pos = 2-m                           # exact
Only two channels of the 12 are ever non-constant: w[idx] = 1-pos and
w[idx+1] = pos; everything else is log(1e-30).  The affine m->pos folds
into the activation's scale/bias (exact, Sterbenz), so the device does
ONE fused bitwise vector op + two activations per element:
    la = Ln(-2^50*m + 2^51)  = ln(1-pos) + 50*ln2
    lb = Ln( 2^50*m - 2^50)  = ln(pos)   + 50*ln2
(the 2^50 scaling keeps the Ln table input inside its accurate range;
the host subtracts 50*ln2).  The host scatters la/lb into a
log(1e-30)-filled (B,S,12) array at channels idx/idx+1 (idx = the same
exponent extraction in numpy), zeroes rows d >= 1024, patches the ~0.1%
of elements with d < 1 (bin [0,1), pos = d does not follow the mantissa
formula) with exact numpy logs, and sets lb = log(1e-30) where pos == 0
exactly (zero mantissa, where the reference's +1e-30 guard matters).

Sharding: batch 128 -> 16 rows per core x 8 cores, each core sees its
shard as [128 partitions x 2048 cols].
"""

import numpy as np

import concourse.bacc as bacc
import concourse.mybir as mybir
import concourse.tile as tile
from concourse.bass_utils import run_bass_kernel_spmd

AF = mybir.ActivationFunctionType
OP = mybir.AluOpType
F32 = mybir.dt.float32
F16 = mybir.dt.float16
I32 = mybir.dt.int32

B, S, K = 128, 16384, 12
NCORES = 8
P = 128
ELEMS = (B // NCORES) * S      # 262144 per core
COLS = ELEMS // P              # 2048
F = 1024                       # free-dim compute tile size
NT = COLS // F                 # compute tiles per core
FIN = 512                      # input dma chunk size
NIN = COLS // FIN

QL = [0.0, 1.0, 2.0, 4.0, 8.0, 16.0, 32.0, 64.0, 128.0, 256.0, 512.0, 1024.0]

# Ln table is only accurate for inputs in ~[1e-19, 1e19]; feed it
# 2^50-scaled weights and subtract 50*ln2 on the host.
LN_SCALE = float(np.float32(2.0**50))
BIAS_LA = float(np.float32(2.0**51))
BIAS_LB = float(np.float32(-(2.0**50)))
LN_OFFSET = np.float32(50.0 * np.log(np.float64(2.0)))
LOG_EPS = np.float32(np.log(np.float64(np.float32(1e-30))))  # -69.07755

MANT_MASK = 0x007FFFFF
ONE_BITS = 0x3F800000


def build_program():
    nc = bacc.Bacc("TRN2", target_bir_lowering=False, debug=False, num_devices=NCORES)
    d_ext = nc.declare_dram_parameter("degrees", [P, COLS], F32, isOutput=False)
    lab_ext = nc.declare_dram_parameter("lab", [P, 2, COLS], F16, isOutput=True)

    with tile.TileContext(nc) as tc:
        with (
            tc.tile_pool(name="dp", bufs=1) as dp,
            tc.tile_pool(name="wp", bufs=2) as wp,
            tc.tile_pool(name="op", bufs=2) as op,
        ):
            # whole-shard input lives in SBUF; DMA it in NIN chunks issued
            # from the sync engine as its first user instructions
            d = dp.tile([P, COLS], F32, tag="d")
            for c in range(NIN):
                nc.sync.dma_start(
                    out=d[:, c * FIN : (c + 1) * FIN],
                    in_=d_ext[:, c * FIN : (c + 1) * FIN],
                )

            # activation bias constants as tracked tiles; memsets on the
            # (nearly idle) vector engine
            cb = dp.tile([P, 2], F32, tag="cb")
            nc.vector.memset(cb[:, 0:1], BIAS_LB)
            nc.vector.memset(cb[:, 1:2], BIAS_LA)
            bias_lb = cb[:, 0:1]
            bias_la = cb[:, 1:2]

            # dummy Ln: pulls the ACT table load into the preamble
            dummy = dp.tile([P, 1], F32, tag="dummy")
            nc.vector.memset(dummy[:], 1.0)
            nc.scalar.activation(dummy[:], dummy[:], AF.Ln, bias=bias_la, scale=-LN_SCALE)

            for t in range(NT):
                ds = d[:, t * F : (t + 1) * F]
                m = wp.tile([P, F], F32, tag="m")
                lab = op.tile([P, 2 * F], F16, tag="lab")

                # m = mantissa(d) in [1,2): one fused bitwise op
                nc.vector.tensor_scalar(
                    m[:].bitcast(I32), ds.bitcast(I32),
                    MANT_MASK, ONE_BITS, OP.bitwise_and, OP.bitwise_or,
                )
                # la = Ln(2^50*(2-m)); lb = Ln(2^50*(m-1))
                nc.scalar.activation(lab[:, :F], m[:], AF.Ln, bias=bias_la, scale=-LN_SCALE)
                nc.scalar.activation(lab[:, F:], m[:], AF.Ln, bias=bias_lb, scale=LN_SCALE)

                nc.sync.dma_start(
                    out=lab_ext[:, :, t * F : (t + 1) * F],
                    in_=lab[:].rearrange("p (c f) -> p c f", c=2),
                )
    nc.compile()
    return nc


_CACHE = {}
RUN_KWARGS = {}  # test harness can set e.g. {"trace": True} for profiling


def kernel(degrees, quantile_values):
    q = np.asarray(quantile_values, dtype=np.float32)
    assert np.array_equal(q, np.array(QL, dtype=np.float32)), "unexpected quantile grid"

    deg = np.ascontiguousarray(np.asarray(degrees, dtype=np.float32)[..., 0])  # (B,S)
    shards = deg.reshape(NCORES, P, COLS)

    if "nc" not in _CACHE:
        _CACHE["nc"] = build_program()
    nc = _CACHE["nc"]

    in_maps = [{"degrees": np.ascontiguousarray(shards[i])} for i in range(NCORES)]
    res = run_bass_kernel_spmd(nc, in_maps, list(range(NCORES)), **RUN_KWARGS)
    _CACHE["last_result"] = res
    labs = np.stack([res.results[i]["lab"] for i in range(NCORES)])  # (8,128,2,2048)

    la = labs[:, :, 0, :].astype(np.float32).reshape(B, S) - LN_OFFSET
    lb = labs[:, :, 1, :].astype(np.float32).reshape(B, S) - LN_OFFSET

    bits = deg.view(np.int32)

    # pos == 0 exactly (zero mantissa): reference's +1e-30 guard -> log(1e-30)
    lb[(bits & MANT_MASK) == 0] = LOG_EPS

    # bin [0,1): device mantissa path doesn't apply; exact host values
    low = deg < np.float32(1.0)
    if low.any():
        dl = deg[low].astype(np.float64)
        la[low] = np.float32(np.log1p(-dl))
        lb[low] = np.float32(np.log(dl + np.float64(np.float32(1e-30))))

    # lo-edge channel: grid is [0, 2^0 .. 2^10], so channel = exponent+1 for
    # d >= 1 and 0 for d < 1; (bits>>23)-126 clipped to [0,10] gives both.
    idx = np.clip((bits >> 23) - 126, 0, 10).astype(np.int64)

    full = np.full((B, S, K), LOG_EPS, dtype=np.float32)
    np.put_along_axis(full, idx[..., None], la[..., None], axis=2)
    np.put_along_axis(full, idx[..., None] + 1, lb[..., None], axis=2)
    full[deg >= np.float32(1024.0)] = np.float32(0.0)
    return full


# revision 10
# speedup vs baseline: 2.5665x; 1.1044x over previous
"""DegreeQuantileConverter Trainium2 kernel (mantissa-trick edition).

deg (B,S,1) f32 -> out (B,S,12) f32 = log(w + 1e-30) where w are the
piecewise-linear interpolation weights of deg onto the quantile grid
q = [0,1,2,4,...,1024], with rows where deg >= 1024 forced to w = 1.

Because the grid is powers of two, for d >= 1 the interpolation position
inside its bin is exactly the f32 mantissa fraction:
    m   = bitcast((bits(d) & 0x7FFFFF) | 0x3F800000)   # in [1,2)
    pos = m - 1,  1-pos = 2-m                           # exact
Only two channels of the 12 are ever non-constant: w[idx] = 1-pos and
w[idx+1] = pos; everything else is log(1e-30).  The affine m->pos folds
into the activation's scale/bias (exact, Sterbenz), so the device does
ONE fused bitwise vector op + two activations per element:
    la = Ln(-2^50*m + 2^51)  = ln(1-pos) + 50*ln2
    lb = Ln( 2^50*m - 2^50)  = ln(pos)   + 50*ln2
(the 2^50 scaling keeps the Ln table input inside its accurate range;
the host subtracts 50*ln2).  The host scatters la/lb into a
log(1e-30)-filled (B,S,12) array at channels idx/idx+1 (idx = the same
exponent extraction in numpy), zeroes rows d >= 1024, patches the ~0.1%
of elements with d < 1 (bin [0,1), pos = d does not follow the mantissa
formula) with exact numpy logs, and sets lb = log(1e-30) where pos == 0
exactly (zero mantissa, where the reference's +1e-30 guard matters).

Sharding: batch 128 -> 16 rows per core x 8 cores, each core sees its
shard as [128 partitions x 2048 cols].
"""

import numpy as np

import concourse.bacc as bacc
import concourse.mybir as mybir
import concourse.tile as tile
from concourse.bass_utils import run_bass_kernel_spmd

AF = mybir.ActivationFunctionType
OP = mybir.AluOpType
F32 = mybir.dt.float32
F16 = mybir.dt.float16
I32 = mybir.dt.int32

B, S, K = 128, 16384, 12
NCORES = 8
P = 128
ELEMS = (B // NCORES) * S      # 262144 per core
COLS = ELEMS // P              # 2048
TILES = [256, 256, 512, 1024]  # progressive tile sizes (sum = COLS)
assert sum(TILES) == COLS

QL = [0.0, 1.0, 2.0, 4.0, 8.0, 16.0, 32.0, 64.0, 128.0, 256.0, 512.0, 1024.0]

# Ln table is only accurate for inputs in ~[1e-19, 1e19]; feed it
# 2^50-scaled weights and subtract 50*ln2 on the host.
LN_SCALE = float(np.float32(2.0**50))
BIAS_LA = float(np.float32(2.0**51))
BIAS_LB = float(np.float32(-(2.0**50)))
LN_OFFSET = np.float32(50.0 * np.log(np.float64(2.0)))
LOG_EPS = np.float32(np.log(np.float64(np.float32(1e-30))))  # -69.07755

MANT_MASK = 0x007FFFFF
ONE_BITS = 0x3F800000


def build_program():
    nc = bacc.Bacc("TRN2", target_bir_lowering=False, debug=False, num_devices=NCORES)
    d_ext = nc.declare_dram_parameter("degrees", [P, COLS], F32, isOutput=False)
    la_ext = nc.declare_dram_parameter("la", [P, COLS], F16, isOutput=True)
    lb_ext = nc.declare_dram_parameter("lb", [P, COLS], F16, isOutput=True)

    with tile.TileContext(nc) as tc:
        with (
            tc.tile_pool(name="dp", bufs=1) as dp,
            tc.tile_pool(name="wp", bufs=2) as wp,
            tc.tile_pool(name="op", bufs=2) as op,
        ):
            # whole-shard input lives in SBUF; DMA it per tile (progressive
            # sizes: first chunks small so compute starts early) issued
            # from the sync engine as its first user instructions
            d = dp.tile([P, COLS], F32, tag="d")
            off = 0
            offs = []
            for f in TILES:
                nc.sync.dma_start(
                    out=d[:, off : off + f],
                    in_=d_ext[:, off : off + f],
                )
                offs.append(off)
                off += f

            # activation bias constants as tracked tiles; memsets on the
            # (nearly idle) vector engine
            cb = dp.tile([P, 2], F32, tag="cb")
            nc.vector.memset(cb[:, 0:1], BIAS_LB)
            nc.vector.memset(cb[:, 1:2], BIAS_LA)
            bias_lb = cb[:, 0:1]
            bias_la = cb[:, 1:2]

            # dummy Ln: pulls the ACT table load into the preamble
            dummy = dp.tile([P, 1], F32, tag="dummy")
            nc.vector.memset(dummy[:], 1.0)
            nc.scalar.activation(dummy[:], dummy[:], AF.Ln, bias=bias_la, scale=-LN_SCALE)

            for t, (f, off) in enumerate(zip(TILES, offs)):
                ds = d[:, off : off + f]
                m = wp.tile([P, f], F32, tag=f"m{t}", name=f"m{t}")
                la32 = wp.tile([P, f], F32, tag=f"la32_{t}", name=f"la32_{t}")
                lb32 = wp.tile([P, f], F32, tag=f"lb32_{t}", name=f"lb32_{t}")
                la16 = op.tile([P, f], F16, tag=f"la16_{t}", name=f"la16_{t}")
                lb16 = op.tile([P, f], F16, tag=f"lb16_{t}", name=f"lb16_{t}")

                # m = mantissa(d) in [1,2): one fused bitwise op
                nc.vector.tensor_scalar(
                    m[:].bitcast(I32), ds.bitcast(I32),
                    MANT_MASK, ONE_BITS, OP.bitwise_and, OP.bitwise_or,
                )
                # la = Ln(2^50*(2-m)); lb = Ln(2^50*(m-1)); DVE converts to f16
                nc.scalar.activation(la32[:], m[:], AF.Ln, bias=bias_la, scale=-LN_SCALE)
                nc.vector.tensor_copy(la16[:], la32[:])
                nc.sync.dma_start(out=la_ext[:, off : off + f], in_=la16[:])
                nc.scalar.activation(lb32[:], m[:], AF.Ln, bias=bias_lb, scale=LN_SCALE)
                nc.vector.tensor_copy(lb16[:], lb32[:])
                nc.sync.dma_start(out=lb_ext[:, off : off + f], in_=lb16[:])
    nc.compile()
    return nc


_CACHE = {}
RUN_KWARGS = {}  # test harness can set e.g. {"trace": True} for profiling


def kernel(degrees, quantile_values):
    q = np.asarray(quantile_values, dtype=np.float32)
    assert np.array_equal(q, np.array(QL, dtype=np.float32)), "unexpected quantile grid"

    deg = np.ascontiguousarray(np.asarray(degrees, dtype=np.float32)[..., 0])  # (B,S)
    shards = deg.reshape(NCORES, P, COLS)

    if "nc" not in _CACHE:
        _CACHE["nc"] = build_program()
    nc = _CACHE["nc"]

    in_maps = [{"degrees": np.ascontiguousarray(shards[i])} for i in range(NCORES)]
    res = run_bass_kernel_spmd(nc, in_maps, list(range(NCORES)), **RUN_KWARGS)
    _CACHE["last_result"] = res
    la = np.stack([res.results[i]["la"] for i in range(NCORES)])  # (8,128,2048) f16
    lb = np.stack([res.results[i]["lb"] for i in range(NCORES)])

    la = la.astype(np.float32).reshape(B, S) - LN_OFFSET
    lb = lb.astype(np.float32).reshape(B, S) - LN_OFFSET

    bits = deg.view(np.int32)

    # pos == 0 exactly (zero mantissa): reference's +1e-30 guard -> log(1e-30)
    lb[(bits & MANT_MASK) == 0] = LOG_EPS

    # bin [0,1): device mantissa path doesn't apply; exact host values
    low = deg < np.float32(1.0)
    if low.any():
        dl = deg[low].astype(np.float64)
        la[low] = np.float32(np.log1p(-dl))
        lb[low] = np.float32(np.log(dl + np.float64(np.float32(1e-30))))

    # lo-edge channel: grid is [0, 2^0 .. 2^10], so channel = exponent+1 for
    # d >= 1 and 0 for d < 1; (bits>>23)-126 clipped to [0,10] gives both.
    idx = np.clip((bits >> 23) - 126, 0, 10).astype(np.int64)

    full = np.full((B, S, K), LOG_EPS, dtype=np.float32)
    np.put_along_axis(full, idx[..., None], la[..., None], axis=2)
    np.put_along_axis(full, idx[..., None] + 1, lb[..., None], axis=2)
    full[deg >= np.float32(1024.0)] = np.float32(0.0)
    return full


# revision 12
# speedup vs baseline: 2.7311x; 1.0641x over previous
"""DegreeQuantileConverter Trainium2 kernel (mantissa-trick edition).

deg (B,S,1) f32 -> out (B,S,12) f32 = log(w + 1e-30) where w are the
piecewise-linear interpolation weights of deg onto the quantile grid
q = [0,1,2,4,...,1024], with rows where deg >= 1024 forced to w = 1.

Because the grid is powers of two, for d >= 1 the interpolation position
inside its bin is exactly the f32 mantissa fraction:
    m   = bitcast((bits(d) & 0x7FFFFF) | 0x3F800000)   # in [1,2)
    pos = m - 1,  1-pos = 2-m                           # exact
Only two channels of the 12 are ever non-constant: w[idx] = 1-pos and
w[idx+1] = pos; everything else is log(1e-30).  The affine m->pos folds
into the activation's scale/bias (exact, Sterbenz), so the device does
ONE fused bitwise vector op + two activations per element:
    la = Ln(-2^50*m + 2^51)  = ln(1-pos) + 50*ln2
    lb = Ln( 2^50*m - 2^50)  = ln(pos)   + 50*ln2
(the 2^50 scaling keeps the Ln table input inside its accurate range;
the host subtracts 50*ln2).  The host scatters la/lb into a
log(1e-30)-filled (B,S,12) array at channels idx/idx+1 (idx = the same
exponent extraction in numpy), zeroes rows d >= 1024, patches the ~0.1%
of elements with d < 1 (bin [0,1), pos = d does not follow the mantissa
formula) with exact numpy logs, and sets lb = log(1e-30) where pos == 0
exactly (zero mantissa, where the reference's +1e-30 guard matters).

Sharding: batch 128 -> 16 rows per core x 8 cores, each core sees its
shard as [128 partitions x 2048 cols].
"""

import numpy as np

import concourse.bacc as bacc
import concourse.mybir as mybir
import concourse.tile as tile
from concourse.bass_utils import run_bass_kernel_spmd

AF = mybir.ActivationFunctionType
OP = mybir.AluOpType
F32 = mybir.dt.float32
F16 = mybir.dt.float16
I32 = mybir.dt.int32

B, S, K = 128, 16384, 12
NCORES = 8
P = 128
ELEMS = (B // NCORES) * S      # 262144 per core
COLS = ELEMS // P              # 2048
TILES = [256, 1024, 512, 256]  # tile sizes: small first (early ACT start)
assert sum(TILES) == COLS      # and small last (short drain tail)

QL = [0.0, 1.0, 2.0, 4.0, 8.0, 16.0, 32.0, 64.0, 128.0, 256.0, 512.0, 1024.0]

# Ln table is only accurate for inputs in ~[1e-19, 1e19]; feed it
# 2^50-scaled weights and subtract 50*ln2 on the host.
LN_SCALE = float(np.float32(2.0**50))
BIAS_LA = float(np.float32(2.0**51))
BIAS_LB = float(np.float32(-(2.0**50)))
LN_OFFSET = np.float32(50.0 * np.log(np.float64(2.0)))
LOG_EPS = np.float32(np.log(np.float64(np.float32(1e-30))))  # -69.07755

MANT_MASK = 0x007FFFFF
ONE_BITS = 0x3F800000


def build_program():
    nc = bacc.Bacc("TRN2", target_bir_lowering=False, debug=False, num_devices=NCORES)
    d_ext = nc.declare_dram_parameter("degrees", [P, COLS], F32, isOutput=False)
    la_ext = nc.declare_dram_parameter("la", [P, COLS], F16, isOutput=True)
    lb_ext = nc.declare_dram_parameter("lb", [P, COLS], F16, isOutput=True)

    with tile.TileContext(nc) as tc:
        with (
            tc.tile_pool(name="dp", bufs=1) as dp,
            tc.tile_pool(name="wp", bufs=2) as wp,
            tc.tile_pool(name="op", bufs=2) as op,
        ):
            # whole-shard input lives in SBUF; DMA it per tile (progressive
            # sizes: first chunks small so compute starts early) issued
            # from the sync engine as its first user instructions
            d = dp.tile([P, COLS], F32, tag="d")
            off = 0
            offs = []
            for f in TILES:
                nc.sync.dma_start(
                    out=d[:, off : off + f],
                    in_=d_ext[:, off : off + f],
                )
                offs.append(off)
                off += f

            # activation bias constants as tracked tiles; memsets on the
            # (nearly idle) vector engine
            cb = dp.tile([P, 2], F32, tag="cb")
            nc.vector.memset(cb[:, 0:1], BIAS_LB)
            nc.vector.memset(cb[:, 1:2], BIAS_LA)
            bias_lb = cb[:, 0:1]
            bias_la = cb[:, 1:2]

            # dummy Ln: pulls the ACT table load into the preamble
            dummy = dp.tile([P, 1], F32, tag="dummy")
            nc.vector.memset(dummy[:], 1.0)
            nc.scalar.activation(dummy[:], dummy[:], AF.Ln, bias=bias_la, scale=-LN_SCALE)

            for t, (f, off) in enumerate(zip(TILES, offs)):
                ds = d[:, off : off + f]
                m = wp.tile([P, f], F32, tag=f"m{t}", name=f"m{t}")
                la16 = op.tile([P, f], F16, tag=f"la16_{t}", name=f"la16_{t}")
                lb16 = op.tile([P, f], F16, tag=f"lb16_{t}", name=f"lb16_{t}")

                # m = mantissa(d) in [1,2): one fused bitwise op
                nc.vector.tensor_scalar(
                    m[:].bitcast(I32), ds.bitcast(I32),
                    MANT_MASK, ONE_BITS, OP.bitwise_and, OP.bitwise_or,
                )
                # la = Ln(2^50*(2-m)); lb = Ln(2^50*(m-1))
                nc.scalar.activation(la16[:], m[:], AF.Ln, bias=bias_la, scale=-LN_SCALE)
                nc.sync.dma_start(out=la_ext[:, off : off + f], in_=la16[:])
                nc.scalar.activation(lb16[:], m[:], AF.Ln, bias=bias_lb, scale=LN_SCALE)
                nc.sync.dma_start(out=lb_ext[:, off : off + f], in_=lb16[:])
    nc.compile()
    return nc


_CACHE = {}
RUN_KWARGS = {}  # test harness can set e.g. {"trace": True} for profiling


def kernel(degrees, quantile_values):
    q = np.asarray(quantile_values, dtype=np.float32)
    assert np.array_equal(q, np.array(QL, dtype=np.float32)), "unexpected quantile grid"

    deg = np.ascontiguousarray(np.asarray(degrees, dtype=np.float32)[..., 0])  # (B,S)
    shards = deg.reshape(NCORES, P, COLS)

    if "nc" not in _CACHE:
        _CACHE["nc"] = build_program()
    nc = _CACHE["nc"]

    in_maps = [{"degrees": np.ascontiguousarray(shards[i])} for i in range(NCORES)]
    res = run_bass_kernel_spmd(nc, in_maps, list(range(NCORES)), **RUN_KWARGS)
    _CACHE["last_result"] = res
    la = np.stack([res.results[i]["la"] for i in range(NCORES)])  # (8,128,2048) f16
    lb = np.stack([res.results[i]["lb"] for i in range(NCORES)])

    la = la.astype(np.float32).reshape(B, S) - LN_OFFSET
    lb = lb.astype(np.float32).reshape(B, S) - LN_OFFSET

    bits = deg.view(np.int32)

    # pos == 0 exactly (zero mantissa): reference's +1e-30 guard -> log(1e-30)
    lb[(bits & MANT_MASK) == 0] = LOG_EPS

    # bin [0,1): device mantissa path doesn't apply; exact host values
    low = deg < np.float32(1.0)
    if low.any():
        dl = deg[low].astype(np.float64)
        la[low] = np.float32(np.log1p(-dl))
        lb[low] = np.float32(np.log(dl + np.float64(np.float32(1e-30))))

    # lo-edge channel: grid is [0, 2^0 .. 2^10], so channel = exponent+1 for
    # d >= 1 and 0 for d < 1; (bits>>23)-126 clipped to [0,10] gives both.
    idx = np.clip((bits >> 23) - 126, 0, 10).astype(np.int64)

    full = np.full((B, S, K), LOG_EPS, dtype=np.float32)
    np.put_along_axis(full, idx[..., None], la[..., None], axis=2)
    np.put_along_axis(full, idx[..., None] + 1, lb[..., None], axis=2)
    full[deg >= np.float32(1024.0)] = np.float32(0.0)
    return full
